# revision 5
# baseline (speedup 1.0000x reference)
"""Trainium2 Bass kernel for nn_CausalAttention (B=2, T=2048, C=2048, H=16, ALiBi).

Sharding: 8 cores = 2 (batch) x 4 (head groups). Core c handles batch c//4 and
heads [g, g+4, g+8, g+12] where g = c%4 (strided so the ALiBi slope mix is
balanced across cores). One SPMD program; every slope-dependent value enters
as data (aug ramps, exp-bias table), never as a program constant.

Per-core device pipeline (matmul operands f32r = fp32 storage, FP22 read):
  A) qT/kT [d,t] and v [t,d] projections from host-pretransposed x^T.
     Wq is host-prescaled by 1/sqrt(D).
  B) Per head: S^T[tk,tq] = kT.T @ qT in PSUM, plus a K=2 "aug" matmul adding
     the ALiBi bias slope*(tk-tq) exactly: small recentred ramps in the aug
     rows plus a per-(kt,j) scalar from a host table applied as the exp bias.
     ACT computes E = exp(.) into SBUF f32r; GPSIMD masks diagonal tiles
     (affine_select, fill 0). PV and the denominator (all-ones matmul whose
     output is the column-sum broadcast across all partitions) accumulate in
     PSUM; DVE computes O_norm^T = O^T * reciprocal(den).
     Far tiles where slope*(tq-tk) >= 150 everywhere are skipped: exp
     underflows to exactly 0 in both this kernel and the fp32 reference.
  C) out[t,c] = sum_h O_norm_h^T.T @ Wo_h accumulated over the 4 local heads.
Host: sums the 4 head-group partials per batch. Key bias bk cancels in
softmax; bv/bo fold into a host-side output bias; bq/bk/bv (zero in practice)
are otherwise added on-device via K=1 outer-product matmuls.
"""

import math
import sys

sys.path.insert(0, "/opt/trn_rl_repo")

import numpy as np

import concourse.mybir as mybir  # noqa: E402
import concourse.tile as tile  # noqa: E402
from concourse import bacc  # noqa: E402
from concourse.bass_utils import run_bass_kernel_spmd  # noqa: E402

B, T, C, H = 2, 2048, 2048, 16
D = C // H  # 128
P = 128
NKC = C // P       # 16 contraction tiles
NKT = T // P       # 16 key tiles
NQC = T // 512     # 4 query chunks of 512
HPG = 4            # heads per core
SQD = math.sqrt(D)
SKIP_CUT = 150.0
F32 = mybir.dt.float32
F32R = mybir.dt.float32r
EXP = mybir.ActivationFunctionType.Exp


def _slopes(n=16):
    start = 2.0 ** (-2.0 ** -(math.log2(n) - 3))
    return [start * start**i for i in range(n)]


SLOPES = _slopes(H)


def _core_heads(g):
    return [g, g + 4, g + 8, g + 12]


def _kts_for_chunk(hi, j):
    # Union over cores: the smallest slope in head-position hi is head 4*hi+3.
    s = SLOPES[4 * hi + 3]
    out = []
    for kt in range(4 * j + 4):
        mind = 512 * j - 128 * kt - 127
        if s * mind < SKIP_CUT:
            out.append(kt)
    return out


_PROG_CACHE = {}


def _build_program(use_b):
    if use_b in _PROG_CACHE:
        return _PROG_CACHE[use_b]
    use_bq, use_bk, use_bv = use_b

    nc = bacc.Bacc(None)
    xt_d = nc.declare_dram_parameter("xt", [C, T], F32R, isOutput=False)
    wq_d = nc.declare_dram_parameter("wq", [C, HPG * D], F32R, isOutput=False)
    wk_d = nc.declare_dram_parameter("wk", [C, HPG * D], F32R, isOutput=False)
    wv_d = nc.declare_dram_parameter("wv", [C, HPG * D], F32R, isOutput=False)
    wo_d = nc.declare_dram_parameter("wo", [HPG * D, C], F32R, isOutput=False)
    qaug_d = nc.declare_dram_parameter("qaug", [HPG, 2, 512], F32R, isOutput=False)
    kaug_d = nc.declare_dram_parameter("kaug", [HPG, 2, P], F32R, isOutput=False)
    ones_d = nc.declare_dram_parameter("ones", [P, P], F32R, isOutput=False)
    btab_d = nc.declare_dram_parameter("btab", [P, HPG * NQC * NKT], F32, isOutput=False)
    if any(use_b):
        bqkv_d = nc.declare_dram_parameter("bqkv", [3, HPG * D], F32R, isOutput=False)
        onesrow_d = nc.declare_dram_parameter("onesrow", [1, 512], F32R, isOutput=False)
    y_d = nc.declare_dram_parameter("y", [T, C], F32, isOutput=True)

    with tile.TileContext(nc) as tc:
        with (
            tc.tile_pool(name="perm", bufs=1) as perm,
            tc.tile_pool(name="dram", bufs=1, space="DRAM") as dpool,
        ):
            ones_sb = perm.tile([P, P], F32R, tag="ones")
            nc.sync.dma_start(ones_sb[:], ones_d[:])
            btab_sb = perm.tile([P, HPG, NQC, NKT], F32, tag="btab")
            nc.sync.dma_start(
                btab_sb[:], btab_d[:].rearrange("p (h j k) -> p h j k", h=HPG, j=NQC)
            )
            onorm = perm.tile([P, HPG, T], F32R, tag="onorm")
            if any(use_b):
                bqkv_sb = perm.tile([3, HPG * D], F32R, tag="bqkv")
                onesrow_sb = perm.tile([1, 512], F32R, tag="onesrow")
                nc.sync.dma_start(bqkv_sb[:], bqkv_d[:])
                nc.sync.dma_start(onesrow_sb[:], onesrow_d[:])

            qt_ds = [dpool.tile([P, T], F32R, tag=f"qtd{h}", name=f"qtd{h}") for h in range(HPG)]
            kt_ds = [dpool.tile([P, T], F32R, tag=f"ktd{h}", name=f"ktd{h}") for h in range(HPG)]
            v_ds = [dpool.tile([P, HPG * D], F32R, tag=f"vd{t}", name=f"vd{t}") for t in range(NKT)]

            # ---------------- Phase A: projections ----------------
            with (
                tc.tile_pool(name="xtp", bufs=2) as xtp,
                tc.tile_pool(name="wp", bufs=1) as wp,
                tc.tile_pool(name="stA", bufs=4) as stA,
                tc.tile_pool(name="psA", bufs=6, space="PSUM") as psA,
            ):
                wq_sb = wp.tile([P, NKC, HPG * D], F32R, tag="wq")
                wk_sb = wp.tile([P, NKC, HPG * D], F32R, tag="wk")
                wv_sb = wp.tile([P, NKC, HPG * D], F32R, tag="wv")
                nc.sync.dma_start(wq_sb[:], wq_d[:].rearrange("(kc p) n -> p kc n", p=P))
                nc.sync.dma_start(wk_sb[:], wk_d[:].rearrange("(kc p) n -> p kc n", p=P))
                nc.sync.dma_start(wv_sb[:], wv_d[:].rearrange("(kc p) n -> p kc n", p=P))

                for tn in range(NQC):
                    ts = slice(tn * 512, (tn + 1) * 512)
                    xt_sb = xtp.tile([P, NKC, 512], F32R, tag="xt")
                    nc.sync.dma_start(
                        xt_sb[:], xt_d[:, ts].rearrange("(kc p) t -> p kc t", p=P)
                    )
                    for w_sb, dsts, ub, brow in (
                        (wq_sb, qt_ds, use_bq, 0),
                        (wk_sb, kt_ds, use_bk, 1),
                    ):
                        for hi in range(HPG):
                            ps = psA.tile([P, 512], F32, tag="pp")
                            for kc in range(NKC):
                                nc.tensor.matmul(
                                    ps[:],
                                    w_sb[:, kc, hi * D:(hi + 1) * D],
                                    xt_sb[:, kc, :],
                                    start=(kc == 0),
                                    stop=(kc == NKC - 1 and not ub),
                                )
                            if ub:
                                nc.tensor.matmul(
                                    ps[:],
                                    bqkv_sb[brow:brow + 1, hi * D:(hi + 1) * D],
                                    onesrow_sb[:],
                                    start=False,
                                    stop=True,
                                )
                            st = stA.tile([P, 512], F32R, tag="st")
                            nc.scalar.copy(st[:], ps[:])
                            nc.sync.dma_start(dsts[hi][:, ts], st[:])
                    for tt in range(4):
                        gt = 4 * tn + tt
                        ps = psA.tile([P, 512], F32, tag="pp")
                        for kc in range(NKC):
                            nc.tensor.matmul(
                                ps[:],
                                xt_sb[:, kc, tt * P:(tt + 1) * P],
                                wv_sb[:, kc, :],
                                start=(kc == 0),
                                stop=(kc == NKC - 1 and not use_bv),
                            )
                        if use_bv:
                            nc.tensor.matmul(
                                ps[:],
                                onesrow_sb[:, :P],
                                bqkv_sb[2:3, :],
                                start=False,
                                stop=True,
                            )
                        st = stA.tile([P, 512], F32R, tag="st")
                        nc.scalar.copy(st[:], ps[:])
                        nc.sync.dma_start(v_ds[gt][:], st[:])

            # ---------------- Phase B: attention ----------------
            with (
                tc.tile_pool(name="hb", bufs=2) as hb,
                tc.tile_pool(name="ep", bufs=2) as ep,
                tc.tile_pool(name="rp", bufs=2) as rp,
                tc.tile_pool(name="psS", bufs=2, space="PSUM") as psS,
                tc.tile_pool(name="psO", bufs=2, space="PSUM") as psO,
                tc.tile_pool(name="psD", bufs=2, space="PSUM") as psD,
            ):
                for hi in range(HPG):
                    qt_sb = hb.tile([P, T], F32R, tag="qt")
                    kt_sb = hb.tile([P, T], F32R, tag="kt")
                    v_sb = hb.tile([P, NKT, D], F32R, tag="v")
                    qa_sb = hb.tile([2, 512], F32R, tag="qa")
                    ka_sb = hb.tile([2, P], F32R, tag="ka")
                    nc.sync.dma_start(qt_sb[:], qt_ds[hi][:])
                    nc.sync.dma_start(kt_sb[:], kt_ds[hi][:])
                    for gt in range(NKT):
                        nc.sync.dma_start(
                            v_sb[:, gt, :], v_ds[gt][:, hi * D:(hi + 1) * D]
                        )
                    nc.sync.dma_start(qa_sb[:], qaug_d[hi])
                    nc.sync.dma_start(ka_sb[:], kaug_d[hi])
                    for j in range(NQC):
                        qs = slice(j * 512, (j + 1) * 512)
                        kts = _kts_for_chunk(hi, j)
                        e_sb = ep.tile([P, NKT, 512], F32R, tag="e")
                        den_ps = psD.tile([P, 512], F32, tag="dp")
                        for idx, kt in enumerate(kts):
                            s_ps = psS.tile([P, 512], F32, tag="sp")
                            nc.tensor.matmul(
                                s_ps[:],
                                kt_sb[:, kt * P:(kt + 1) * P],
                                qt_sb[:, qs],
                                start=True,
                                stop=False,
                            )
                            nc.tensor.matmul(
                                s_ps[:], ka_sb[:], qa_sb[:], start=False, stop=True
                            )
                            nc.scalar.activation(
                                e_sb[:, idx, :],
                                s_ps[:],
                                EXP,
                                bias=btab_sb[:, hi, j, kt:kt + 1],
                                scale=1.0,
                            )
                            if 128 * kt > 512 * j - 128:  # diagonal-crossing tile
                                nc.gpsimd.affine_select(
                                    e_sb[:, idx, :],
                                    e_sb[:, idx, :],
                                    pattern=[[1, 512]],
                                    compare_op=mybir.AluOpType.is_ge,
                                    fill=0.0,
                                    base=512 * j - 128 * kt,
                                    channel_multiplier=-1,
                                )
                            nc.tensor.matmul(
                                den_ps[:],
                                ones_sb[:],
                                e_sb[:, idx, :],
                                start=(idx == 0),
                                stop=(idx == len(kts) - 1),
                            )
                        o_ps = psO.tile([P, 512], F32, tag="op")
                        for idx, kt in enumerate(kts):
                            nc.tensor.matmul(
                                o_ps[:],
                                v_sb[:, kt, :],
                                e_sb[:, idx, :],
                                start=(idx == 0),
                                stop=(idx == len(kts) - 1),
                            )
                        rec = rp.tile([P, 512], F32, tag="rec")
                        nc.vector.reciprocal(rec[:], den_ps[:])
                        nc.vector.tensor_mul(onorm[:, hi, qs], o_ps[:], rec[:])

            # ---------------- Phase C: output projection ----------------
            with (
                tc.tile_pool(name="wop", bufs=1) as wop,
                tc.tile_pool(name="stC", bufs=4) as stC,
                tc.tile_pool(name="psC", bufs=6, space="PSUM") as psC,
            ):
                wo_sb = wop.tile([P, HPG, C], F32R, tag="wo")
                nc.sync.dma_start(
                    wo_sb[:], wo_d[:].rearrange("(h p) c -> p h c", p=P)
                )
                for tt in range(NKT):
                    for cn in range(NQC):
                        ps = psC.tile([P, 512], F32, tag="pc")
                        for hi in range(HPG):
                            nc.tensor.matmul(
                                ps[:],
                                onorm[:, hi, tt * P:(tt + 1) * P],
                                wo_sb[:, hi, cn * 512:(cn + 1) * 512],
                                start=(hi == 0),
                                stop=(hi == HPG - 1),
                            )
                        st = stC.tile([P, 512], F32, tag="st")
                        nc.scalar.copy(st[:], ps[:])
                        nc.sync.dma_start(
                            y_d[tt * P:(tt + 1) * P, cn * 512:(cn + 1) * 512], st[:]
                        )

    nc.compile()
    _PROG_CACHE[use_b] = nc
    return nc


def _host_inputs(x, Wq, bq, Wk, bk, Wv, bv, Wo, bo, use_b):
    """Build the 8 per-core input maps."""
    x = np.asarray(x, np.float32)
    Wq = np.asarray(Wq, np.float32)
    Wk = np.asarray(Wk, np.float32)
    Wv = np.asarray(Wv, np.float32)
    Wo = np.asarray(Wo, np.float32)
    bq = np.asarray(bq, np.float32)
    bk = np.asarray(bk, np.float32)
    bv = np.asarray(bv, np.float32)

    ones = np.ones((P, P), np.float32)
    onesrow = np.ones((1, 512), np.float32)
    in_maps = []
    for c in range(8):
        b, g = divmod(c, 4)
        heads = _core_heads(g)
        cols = np.concatenate([np.arange(h * D, (h + 1) * D) for h in heads])
        xt = np.ascontiguousarray(x[b].T)
        wq = np.ascontiguousarray(Wq[:, cols]) * np.float32(1.0 / SQD)
        wk = np.ascontiguousarray(Wk[:, cols])
        wv = np.ascontiguousarray(Wv[:, cols])
        wo = np.ascontiguousarray(Wo[cols, :])

        qaug = np.zeros((HPG, 2, 512), np.float32)
        kaug = np.zeros((HPG, 2, P), np.float32)
        btab = np.zeros((P, HPG, NQC, NKT), np.float32)
        f = np.arange(512, dtype=np.float64)
        p = np.arange(P, dtype=np.float64)
        for hi, h in enumerate(heads):
            s = SLOPES[h]
            kaug[hi, 0] = (s * (p - 64)).astype(np.float32)
            kaug[hi, 1] = 1.0
            qaug[hi, 0] = 1.0
            qaug[hi, 1] = (-s * (f - 256)).astype(np.float32)
            for j in range(NQC):
                for kt in range(NKT):
                    btab[:, hi, j, kt] = np.float32(s * (128 * kt - 512 * j - 192))
        m = {
            "xt": xt, "wq": wq, "wk": wk, "wv": wv, "wo": wo,
            "qaug": qaug, "kaug": kaug, "ones": ones,
            "btab": btab.reshape(P, HPG * NQC * NKT),
        }
        if any(use_b):
            bqkv = np.stack([
                bq[cols] * np.float32(1.0 / SQD), bk[cols], bv[cols]
            ]).astype(np.float32)
            m["bqkv"] = bqkv
            m["onesrow"] = onesrow
        in_maps.append(m)
    return in_maps


def _gather(results, bo):
    out = np.zeros((B, T, C), np.float32)
    for c in range(8):
        b = c // 4
        out[b] += results[c]["y"]
    out += np.asarray(bo, np.float32)[None, None, :]
    return out


def run(inputs, trace=False, tmpdir=None, trace_cores=None):
    """Full pipeline; returns (output, BassKernelResults)."""
    x = inputs["x"]
    use_b = (
        bool(np.any(inputs["bq"])),
        bool(np.any(inputs["bk"])),
        bool(np.any(inputs["bv"])),
    )
    nc = _build_program(use_b)
    in_maps = _host_inputs(
        x, inputs["Wq"], inputs["bq"], inputs["Wk"], inputs["bk"],
        inputs["Wv"], inputs["bv"], inputs["Wo"], inputs["bo"], use_b
    )
    res = run_bass_kernel_spmd(
        nc, in_maps, list(range(8)), trace=trace, tmpdir=tmpdir,
        trace_cores=trace_cores,
    )
    out = _gather(res.results, inputs["bo"])
    return out, res


def kernel(**inputs):
    out, _ = run(inputs, trace=False)
    return out


# revision 10
# speedup vs baseline: 1.0638x; 1.0638x over previous
"""Trainium2 Bass kernel for nn_CausalAttention (B=2, T=2048, C=2048, H=16, ALiBi).

Sharding: 8 cores = 2 (batch) x 4 (head groups). Core c handles batch c//4 and
heads [g, g+4, g+8, g+12] where g = c%4 (strided so the ALiBi slope mix is
balanced across cores). One SPMD program; every slope-dependent value enters
as data (aug ramps, exp-bias table), never as a program constant.

Per-core device pipeline (matmul operands f32r = fp32 storage, FP22 read):
  A) qT/kT [d,t] and v [t,d] projections from host-pretransposed x^T.
     Wq is host-prescaled by 1/sqrt(D).
  B) Per head: S^T[tk,tq] = kT.T @ qT in PSUM, plus a K=2 "aug" matmul adding
     the ALiBi bias slope*(tk-tq) exactly: small recentred ramps in the aug
     rows plus a per-(kt,j) scalar from a host table applied as the exp bias.
     ACT computes E = exp(.) into SBUF f32r; GPSIMD masks diagonal tiles
     (affine_select, fill 0). PV and the denominator (all-ones matmul whose
     output is the column-sum broadcast across all partitions) accumulate in
     PSUM; DVE computes O_norm^T = O^T * reciprocal(den).
     Far tiles where slope*(tq-tk) >= 150 everywhere are skipped: exp
     underflows to exactly 0 in both this kernel and the fp32 reference.
  C) out[t,c] = sum_h O_norm_h^T.T @ Wo_h accumulated over the 4 local heads.
Host: sums the 4 head-group partials per batch. Key bias bk cancels in
softmax; bv/bo fold into a host-side output bias; bq/bk/bv (zero in practice)
are otherwise added on-device via K=1 outer-product matmuls.
"""

import math
import sys

sys.path.insert(0, "/opt/trn_rl_repo")

import numpy as np

import concourse.mybir as mybir  # noqa: E402
import concourse.tile as tile  # noqa: E402
from concourse import bacc  # noqa: E402
from concourse.bass_utils import run_bass_kernel_spmd  # noqa: E402

B, T, C, H = 2, 2048, 2048, 16
D = C // H  # 128
P = 128
NKC = C // P       # 16 contraction tiles
NKT = T // P       # 16 key tiles
NQC = T // 512     # 4 query chunks of 512
HPG = 4            # heads per core
SQD = math.sqrt(D)
SKIP_CUT = 150.0
F32 = mybir.dt.float32
F32R = mybir.dt.float32r
EXP = mybir.ActivationFunctionType.Exp


def _slopes(n=16):
    start = 2.0 ** (-2.0 ** -(math.log2(n) - 3))
    return [start * start**i for i in range(n)]


SLOPES = _slopes(H)


def _core_heads(g):
    return [g, g + 4, g + 8, g + 12]


def _kts_for_chunk(hi, j):
    # Union over cores: the smallest slope in head-position hi is head 4*hi+3.
    s = SLOPES[4 * hi + 3]
    out = []
    for kt in range(4 * j + 4):
        mind = 512 * j - 128 * kt - 127
        if s * mind < SKIP_CUT:
            out.append(kt)
    return out


_PROG_CACHE = {}


def _build_program(use_b):
    if use_b in _PROG_CACHE:
        return _PROG_CACHE[use_b]
    use_bq, use_bk, use_bv = use_b

    nc = bacc.Bacc(None)
    xt_d = nc.declare_dram_parameter("xt", [C, T], F32R, isOutput=False)
    wq_d = nc.declare_dram_parameter("wq", [C, HPG * D], F32R, isOutput=False)
    wk_d = nc.declare_dram_parameter("wk", [C, HPG * D], F32R, isOutput=False)
    wv_d = nc.declare_dram_parameter("wv", [C, HPG * D], F32R, isOutput=False)
    wo_d = nc.declare_dram_parameter("wo", [HPG * D, C], F32R, isOutput=False)
    qaug_d = nc.declare_dram_parameter("qaug", [HPG, 2, 512], F32R, isOutput=False)
    kaug_d = nc.declare_dram_parameter("kaug", [HPG, 2, P], F32R, isOutput=False)
    ones_d = nc.declare_dram_parameter("ones", [P, P], F32R, isOutput=False)
    btab_d = nc.declare_dram_parameter("btab", [P, HPG * NQC * NKT], F32, isOutput=False)
    if any(use_b):
        bqkv_d = nc.declare_dram_parameter("bqkv", [3, HPG * D], F32R, isOutput=False)
        onesrow_d = nc.declare_dram_parameter("onesrow", [1, 512], F32R, isOutput=False)
    y_d = nc.declare_dram_parameter("y", [T, C], F32, isOutput=True)

    with tile.TileContext(nc) as tc:
        with (
            tc.tile_pool(name="perm", bufs=1) as perm,
            tc.tile_pool(name="dram", bufs=1, space="DRAM") as dpool,
        ):
            ones_sb = perm.tile([P, P], F32R, tag="ones")
            nc.sync.dma_start(ones_sb[:], ones_d[:])
            btab_sb = perm.tile([P, HPG, NQC, NKT], F32, tag="btab")
            nc.sync.dma_start(
                btab_sb[:], btab_d[:].rearrange("p (h j k) -> p h j k", h=HPG, j=NQC)
            )
            onorm = perm.tile([P, HPG, T], F32R, tag="onorm")
            if any(use_b):
                bqkv_sb = perm.tile([3, HPG * D], F32R, tag="bqkv")
                onesrow_sb = perm.tile([1, 512], F32R, tag="onesrow")
                nc.sync.dma_start(bqkv_sb[:], bqkv_d[:])
                nc.sync.dma_start(onesrow_sb[:], onesrow_d[:])

            qt_ds = [dpool.tile([P, T], F32R, tag=f"qtd{h}", name=f"qtd{h}") for h in range(HPG)]
            kt_ds = [dpool.tile([P, T], F32R, tag=f"ktd{h}", name=f"ktd{h}") for h in range(HPG)]
            v_ds = [dpool.tile([P, HPG * D], F32R, tag=f"vd{t}", name=f"vd{t}") for t in range(NKT)]

            # ---------------- Phase A: projections ----------------
            with (
                tc.tile_pool(name="xtp", bufs=2) as xtp,
                tc.tile_pool(name="wp", bufs=1) as wp,
                tc.tile_pool(name="stA", bufs=4) as stA,
                tc.tile_pool(name="psA", bufs=6, space="PSUM") as psA,
            ):
                wq_sb = wp.tile([P, NKC, HPG * D], F32R, tag="wq")
                wk_sb = wp.tile([P, NKC, HPG * D], F32R, tag="wk")
                wv_sb = wp.tile([P, NKC, HPG * D], F32R, tag="wv")
                # Per-kc loads so the first matmuls unblock after ~1 small DMA
                # instead of a serialized 12 MB weight load.
                for kc in range(NKC):
                    nc.sync.dma_start(
                        wq_sb[:, kc, :], wq_d[kc * P:(kc + 1) * P, :]
                    )
                for kc in range(NKC):
                    nc.sync.dma_start(
                        wk_sb[:, kc, :], wk_d[kc * P:(kc + 1) * P, :]
                    )
                for kc in range(NKC):
                    nc.sync.dma_start(
                        wv_sb[:, kc, :], wv_d[kc * P:(kc + 1) * P, :]
                    )

                for tn in range(NQC):
                    ts = slice(tn * 512, (tn + 1) * 512)
                    xt_sb = xtp.tile([P, NKC, 512], F32R, tag="xt")
                    for kc in range(NKC):
                        nc.sync.dma_start(
                            xt_sb[:, kc, :], xt_d[kc * P:(kc + 1) * P, ts]
                        )
                    for w_sb, dsts, ub, brow in (
                        (wq_sb, qt_ds, use_bq, 0),
                        (wk_sb, kt_ds, use_bk, 1),
                    ):
                        for hi in range(HPG):
                            ps = psA.tile([P, 512], F32, tag="pp")
                            for kc in range(NKC):
                                nc.tensor.matmul(
                                    ps[:],
                                    w_sb[:, kc, hi * D:(hi + 1) * D],
                                    xt_sb[:, kc, :],
                                    start=(kc == 0),
                                    stop=(kc == NKC - 1 and not ub),
                                )
                            if ub:
                                nc.tensor.matmul(
                                    ps[:],
                                    bqkv_sb[brow:brow + 1, hi * D:(hi + 1) * D],
                                    onesrow_sb[:],
                                    start=False,
                                    stop=True,
                                )
                            st = stA.tile([P, 512], F32R, tag="st")
                            nc.vector.tensor_copy(st[:], ps[:])
                            nc.sync.dma_start(dsts[hi][:, ts], st[:])
                    for tt in range(4):
                        gt = 4 * tn + tt
                        ps = psA.tile([P, 512], F32, tag="pp")
                        for kc in range(NKC):
                            nc.tensor.matmul(
                                ps[:],
                                xt_sb[:, kc, tt * P:(tt + 1) * P],
                                wv_sb[:, kc, :],
                                start=(kc == 0),
                                stop=(kc == NKC - 1 and not use_bv),
                            )
                        if use_bv:
                            nc.tensor.matmul(
                                ps[:],
                                onesrow_sb[:, :P],
                                bqkv_sb[2:3, :],
                                start=False,
                                stop=True,
                            )
                        st = stA.tile([P, 512], F32R, tag="st")
                        nc.vector.tensor_copy(st[:], ps[:])
                        nc.sync.dma_start(v_ds[gt][:], st[:])

            # ---------------- Phase B: attention ----------------
            # wo prefetch: pool opened before phase B so the 4 MB load
            # overlaps attention instead of stalling phase C.
            wop = tc.alloc_tile_pool(name="wop", bufs=1)
            wo_sb = wop.tile([P, HPG, C], F32R, tag="wo")
            for h in range(HPG):
                nc.sync.dma_start(
                    wo_sb[:, h, :], wo_d[h * P:(h + 1) * P, :]
                )
            with (
                tc.tile_pool(name="hb", bufs=2) as hb,
                tc.tile_pool(name="ep", bufs=2) as ep,
                tc.tile_pool(name="rp", bufs=2) as rp,
                tc.tile_pool(name="psS", bufs=2, space="PSUM") as psS,
                tc.tile_pool(name="psO", bufs=2, space="PSUM") as psO,
                tc.tile_pool(name="psD", bufs=2, space="PSUM") as psD,
            ):
                for hi in range(HPG):
                    qt_sb = hb.tile([P, T], F32R, tag="qt")
                    kt_sb = hb.tile([P, T], F32R, tag="kt")
                    v_sb = hb.tile([P, NKT, D], F32R, tag="v")
                    qa_sb = hb.tile([2, 512], F32R, tag="qa")
                    ka_sb = hb.tile([2, P], F32R, tag="ka")
                    for sl in range(NQC):
                        ss = slice(sl * 512, (sl + 1) * 512)
                        nc.sync.dma_start(qt_sb[:, ss], qt_ds[hi][:, ss])
                        nc.sync.dma_start(kt_sb[:, ss], kt_ds[hi][:, ss])
                    for gt in range(NKT):
                        nc.sync.dma_start(
                            v_sb[:, gt, :], v_ds[gt][:, hi * D:(hi + 1) * D]
                        )
                    nc.sync.dma_start(qa_sb[:], qaug_d[hi])
                    nc.sync.dma_start(ka_sb[:], kaug_d[hi])
                    for j in range(NQC):
                        qs = slice(j * 512, (j + 1) * 512)
                        kts = _kts_for_chunk(hi, j)
                        e_sb = ep.tile([P, NKT, 512], F32R, tag="e")
                        den_ps = psD.tile([P, 512], F32, tag="dp")
                        for idx, kt in enumerate(kts):
                            s_ps = psS.tile([P, 512], F32, tag="sp")
                            nc.tensor.matmul(
                                s_ps[:],
                                kt_sb[:, kt * P:(kt + 1) * P],
                                qt_sb[:, qs],
                                start=True,
                                stop=False,
                            )
                            nc.tensor.matmul(
                                s_ps[:], ka_sb[:], qa_sb[:], start=False, stop=True
                            )
                            nc.scalar.activation(
                                e_sb[:, idx, :],
                                s_ps[:],
                                EXP,
                                bias=btab_sb[:, hi, j, kt:kt + 1],
                                scale=1.0,
                            )
                            if 128 * kt > 512 * j - 128:  # diagonal-crossing tile
                                nc.gpsimd.affine_select(
                                    e_sb[:, idx, :],
                                    e_sb[:, idx, :],
                                    pattern=[[1, 512]],
                                    compare_op=mybir.AluOpType.is_ge,
                                    fill=0.0,
                                    base=512 * j - 128 * kt,
                                    channel_multiplier=-1,
                                )
                            nc.tensor.matmul(
                                den_ps[:],
                                ones_sb[:],
                                e_sb[:, idx, :],
                                start=(idx == 0),
                                stop=(idx == len(kts) - 1),
                            )
                        o_ps = psO.tile([P, 512], F32, tag="op")
                        for idx, kt in enumerate(kts):
                            nc.tensor.matmul(
                                o_ps[:],
                                v_sb[:, kt, :],
                                e_sb[:, idx, :],
                                start=(idx == 0),
                                stop=(idx == len(kts) - 1),
                            )
                        rec = rp.tile([P, 512], F32, tag="rec")
                        nc.vector.reciprocal_approx_fast(rec[:], den_ps[:])
                        nc.vector.tensor_mul(onorm[:, hi, qs], o_ps[:], rec[:])

            # ---------------- Phase C: output projection ----------------
            with (
                tc.tile_pool(name="stC", bufs=4) as stC,
                tc.tile_pool(name="psC", bufs=6, space="PSUM") as psC,
            ):
                for tt in range(NKT):
                    for cn in range(NQC):
                        ps = psC.tile([P, 512], F32, tag="pc")
                        for hi in range(HPG):
                            nc.tensor.matmul(
                                ps[:],
                                onorm[:, hi, tt * P:(tt + 1) * P],
                                wo_sb[:, hi, cn * 512:(cn + 1) * 512],
                                start=(hi == 0),
                                stop=(hi == HPG - 1),
                            )
                        st = stC.tile([P, 512], F32, tag="st")
                        nc.vector.tensor_copy(st[:], ps[:])
                        nc.sync.dma_start(
                            y_d[tt * P:(tt + 1) * P, cn * 512:(cn + 1) * 512], st[:]
                        )
            wop.release()

    nc.compile()
    _PROG_CACHE[use_b] = nc
    return nc


def _host_inputs(x, Wq, bq, Wk, bk, Wv, bv, Wo, bo, use_b):
    """Build the 8 per-core input maps."""
    x = np.asarray(x, np.float32)
    Wq = np.asarray(Wq, np.float32)
    Wk = np.asarray(Wk, np.float32)
    Wv = np.asarray(Wv, np.float32)
    Wo = np.asarray(Wo, np.float32)
    bq = np.asarray(bq, np.float32)
    bk = np.asarray(bk, np.float32)
    bv = np.asarray(bv, np.float32)

    ones = np.ones((P, P), np.float32)
    onesrow = np.ones((1, 512), np.float32)
    in_maps = []
    for c in range(8):
        b, g = divmod(c, 4)
        heads = _core_heads(g)
        cols = np.concatenate([np.arange(h * D, (h + 1) * D) for h in heads])
        xt = np.ascontiguousarray(x[b].T)
        wq = np.ascontiguousarray(Wq[:, cols]) * np.float32(1.0 / SQD)
        wk = np.ascontiguousarray(Wk[:, cols])
        wv = np.ascontiguousarray(Wv[:, cols])
        wo = np.ascontiguousarray(Wo[cols, :])

        qaug = np.zeros((HPG, 2, 512), np.float32)
        kaug = np.zeros((HPG, 2, P), np.float32)
        btab = np.zeros((P, HPG, NQC, NKT), np.float32)
        f = np.arange(512, dtype=np.float64)
        p = np.arange(P, dtype=np.float64)
        for hi, h in enumerate(heads):
            s = SLOPES[h]
            kaug[hi, 0] = (s * (p - 64)).astype(np.float32)
            kaug[hi, 1] = 1.0
            qaug[hi, 0] = 1.0
            qaug[hi, 1] = (-s * (f - 256)).astype(np.float32)
            for j in range(NQC):
                for kt in range(NKT):
                    btab[:, hi, j, kt] = np.float32(s * (128 * kt - 512 * j - 192))
        m = {
            "xt": xt, "wq": wq, "wk": wk, "wv": wv, "wo": wo,
            "qaug": qaug, "kaug": kaug, "ones": ones,
            "btab": btab.reshape(P, HPG * NQC * NKT),
        }
        if any(use_b):
            bqkv = np.stack([
                bq[cols] * np.float32(1.0 / SQD), bk[cols], bv[cols]
            ]).astype(np.float32)
            m["bqkv"] = bqkv
            m["onesrow"] = onesrow
        in_maps.append(m)
    return in_maps


def _gather(results, bo):
    out = np.zeros((B, T, C), np.float32)
    for c in range(8):
        b = c // 4
        out[b] += results[c]["y"]
    out += np.asarray(bo, np.float32)[None, None, :]
    return out


def run(inputs, trace=False, tmpdir=None, trace_cores=None):
    """Full pipeline; returns (output, BassKernelResults)."""
    x = inputs["x"]
    use_b = (
        bool(np.any(inputs["bq"])),
        bool(np.any(inputs["bk"])),
        bool(np.any(inputs["bv"])),
    )
    nc = _build_program(use_b)
    in_maps = _host_inputs(
        x, inputs["Wq"], inputs["bq"], inputs["Wk"], inputs["bk"],
        inputs["Wv"], inputs["bv"], inputs["Wo"], inputs["bo"], use_b
    )
    res = run_bass_kernel_spmd(
        nc, in_maps, list(range(8)), trace=trace, tmpdir=tmpdir,
        trace_cores=trace_cores,
    )
    out = _gather(res.results, inputs["bo"])
    return out, res


def kernel(**inputs):
    out, _ = run(inputs, trace=False)
    return out
